# revision 1
# baseline (speedup 1.0000x reference)
"""Trainium2 Bass kernel for causal multi-head attention with RoPE.

Full-input contract: kernel(**inputs) takes the unsharded tensors and
returns the full [B, S, D] output. Internally the work is sharded over
8 NeuronCores: cores 0-3 compute batch 0, cores 4-7 batch 1; within a
batch group each core owns 4 of the 16 heads (tensor-parallel over
heads). Each core computes its partial output-projection contribution
[S, D]; the host sums the 4 partials per batch and adds the biases
that commute with attention (wo_b, and wv_b which passes through the
softmax untouched because attention weights sum to 1).

Matmuls run in float32r (hardware rounds operands to ~11 mantissa
bits, fp32 accumulate in PSUM) at 4x the fp32 rate.
"""

import os
import sys

sys.path.insert(0, "/opt/trn_rl_repo")

import numpy as np

B = 2
S = 2048
D = 2048
H = 16
DK = 128
N_CORES = 8
HPC = 4          # heads per core
E = HPC * DK     # 512: per-core slice of the model dim
AN = 256         # phase-A sequence chunk (moving free dim for Q/K)
SC = 512         # attention query chunk (moving free dim)
KO = D // 128    # contraction chunks for the projections
NJ = S // 128    # key chunks
NI = S // SC     # query chunks
ISQRT_DK = 1.0 / np.sqrt(DK)

_CACHE = {}

last_exec_time_ns = None
last_results = None


def _build_program():
    import concourse.mybir as mybir
    import concourse.tile as tile
    from concourse import bacc

    dt = mybir.dt
    F32 = dt.float32
    F32R = dt.float32r
    AF = mybir.ActivationFunctionType

    nc = bacc.Bacc(None, target_bir_lowering=False, debug=True)

    xT = nc.dram_tensor("xT", [D, S], F32R, kind="ExternalInput")
    wqT = nc.dram_tensor("wqT", [D, E], F32R, kind="ExternalInput")
    wkT = nc.dram_tensor("wkT", [D, E], F32R, kind="ExternalInput")
    wvT = nc.dram_tensor("wvT", [D, E], F32R, kind="ExternalInput")
    woT = nc.dram_tensor("woT", [E, D], F32R, kind="ExternalInput")
    bq = nc.dram_tensor("bq", [HPC, DK], F32, kind="ExternalInput")
    bk = nc.dram_tensor("bk", [HPC, DK], F32, kind="ExternalInput")
    cc2 = nc.dram_tensor("cc2", [DK, S], F32R, kind="ExternalInput")
    sss = nc.dram_tensor("sss", [DK, S], F32R, kind="ExternalInput")
    masks = nc.dram_tensor("masks", [HPC, 128, SC], F32R, kind="ExternalInput")
    ones = nc.dram_tensor("ones", [128, 128], F32R, kind="ExternalInput")
    out = nc.dram_tensor("out", [S, D], F32, kind="ExternalOutput")

    with tile.TileContext(nc) as tc:
        with (
            tc.tile_pool(name="dram", bufs=1, space="DRAM") as dpool,
            tc.tile_pool(name="const", bufs=1) as cpool,
        ):
            q_d = dpool.tile([HPC, DK, S], F32R, name="q_d")
            k_d = dpool.tile([HPC, DK, S], F32R, name="k_d")

            bq_sb = cpool.tile([DK, HPC], F32, name="bq_sb")
            nc.sync.dma_start(bq_sb[:], bq[:].rearrange("h d -> d h"))
            bk_sb = cpool.tile([DK, HPC], F32, name="bk_sb")
            nc.sync.dma_start(bk_sb[:], bk[:].rearrange("h d -> d h"))
            cc2_sb = cpool.tile([DK, S], F32R, name="cc2_sb")
            nc.gpsimd.dma_start(cc2_sb[:], cc2[:])
            sss_sb = cpool.tile([DK, S], F32R, name="sss_sb")
            nc.gpsimd.dma_start(sss_sb[:], sss[:])

            # V stays resident in SBUF from the projection through attention
            vres_ctx = tc.tile_pool(name="vres", bufs=1)
            vrpool = vres_ctx.__enter__()
            vt_all = vrpool.tile([128, NJ, E], F32R, name="vt_all")

            # ---------- Phase A: Q/K/V projections (+ RoPE on Q/K) ----------
            with (
                tc.tile_pool(name="aw", bufs=1) as awpool,
                tc.tile_pool(name="ax", bufs=2) as axpool,
                tc.tile_pool(name="ast", bufs=2) as astpool,
                tc.tile_pool(name="aso", bufs=3) as asopool,
                tc.tile_pool(name="aps", bufs=2, space="PSUM") as apspool,
            ):
                # per-k-chunk DMAs so the first matmuls start as soon as the
                # first 256KB pieces land (instead of after whole-tensor DMAs)
                def load_xn(n):
                    xn = axpool.tile([128, KO, AN], F32R, tag="xn", name=f"xn{n}")
                    for g in range(4):
                        nc.sync.dma_start(
                            xn[:, g * 4 : (g + 1) * 4, :],
                            xT[
                                g * 512 : (g + 1) * 512,
                                n * AN : (n + 1) * AN,
                            ].rearrange("(ko p) s -> p ko s", p=128),
                        )
                    return xn

                # strictly need-ordered input stream on one queue: the first
                # Q chain consumes (wq, x0) k-group pairs, then K needs wk,
                # then V needs wv; later x chunks stream behind
                wq_sb = awpool.tile([128, KO, E], F32R, name="wq_sb")
                wk_sb = awpool.tile([128, KO, E], F32R, name="wk_sb")
                wv_sb = awpool.tile([128, KO, E], F32R, name="wv_sb")
                xn_next = axpool.tile([128, KO, AN], F32R, tag="xn", name="xn0")
                for g in range(4):
                    nc.sync.dma_start(
                        wq_sb[:, g * 4 : (g + 1) * 4, :],
                        wqT[g * 512 : (g + 1) * 512, :].rearrange(
                            "(ko p) m -> p ko m", p=128
                        ),
                    )
                    nc.sync.dma_start(
                        xn_next[:, g * 4 : (g + 1) * 4, :],
                        xT[g * 512 : (g + 1) * 512, 0:AN].rearrange(
                            "(ko p) s -> p ko s", p=128
                        ),
                    )
                for wsb, wdram in ((wk_sb, wkT), (wv_sb, wvT)):
                    for g in range(4):
                        nc.sync.dma_start(
                            wsb[:, g * 4 : (g + 1) * 4, :],
                            wdram[g * 512 : (g + 1) * 512, :].rearrange(
                                "(ko p) m -> p ko m", p=128
                            ),
                        )

                for n in range(S // AN):
                    xn = xn_next
                    if n + 1 < S // AN:
                        xn_next = load_xn(n + 1)
                    nsl = slice(n * AN, (n + 1) * AN)
                    # Q and K: out[d, s], then bias + RoPE here (DVE is idle
                    # during the projections; keeps attention phase lean).
                    # First chunk runs k-outer so the PE consumes each weight/x
                    # k-group as it arrives instead of stalling mid-chain.
                    for wsb, bsb, dst in ((wq_sb, bq_sb, q_d), (wk_sb, bk_sb, k_d)):
                        if n == 0:
                            pqs = [
                                apspool.tile(
                                    [128, AN], F32, tag="pqk0", name=f"pq0_{m}"
                                )
                                for m in range(HPC)
                            ]
                            for k in range(KO):
                                for m in range(HPC):
                                    nc.tensor.matmul(
                                        pqs[m][:],
                                        wsb[:, k, m * DK : (m + 1) * DK],
                                        xn[:, k, :],
                                        start=(k == 0),
                                        stop=(k == KO - 1),
                                    )
                        for m in range(HPC):
                            if n == 0:
                                pq = pqs[m]
                            else:
                                pq = apspool.tile([128, AN], F32, tag="pqk")
                                for k in range(KO):
                                    nc.tensor.matmul(
                                        pq[:],
                                        wsb[:, k, m * DK : (m + 1) * DK],
                                        xn[:, k, :],
                                        start=(k == 0),
                                        stop=(k == KO - 1),
                                    )
                            st0 = astpool.tile([128, AN], F32, tag="qkst0")
                            nc.scalar.activation(
                                st0[:], pq[:], AF.Identity, bias=bsb[:, m : m + 1]
                            )
                            # RoPE: d-rows are packed [even; odd] per head, so
                            # rotate pairs are partition r <-> r+64
                            sw = astpool.tile([128, AN], F32, tag="qksw")
                            nc.vector.tensor_copy(sw[0:64, :], st0[64:128, :])
                            nc.vector.tensor_copy(sw[64:128, :], st0[0:64, :])
                            rot = asopool.tile([128, AN], F32R, tag="stout", name="rot")
                            nc.vector.tensor_mul(rot[:], st0[:], cc2_sb[:, nsl])
                            nc.vector.tensor_mul(sw[:], sw[:], sss_sb[:, nsl])
                            nc.vector.tensor_add(rot[:], rot[:], sw[:])
                            nc.scalar.dma_start(dst[m, :, nsl], rot[:])
                    # V: out[s, d] with s on partitions (natural for P@V)
                    for jj in range(AN // 128):
                        pv = apspool.tile([128, E], F32, tag="pv")
                        for k in range(KO):
                            nc.tensor.matmul(
                                pv[:],
                                xn[:, k, jj * 128 : (jj + 1) * 128],
                                wv_sb[:, k, :],
                                start=(k == 0),
                                stop=(k == KO - 1),
                            )
                        jc_g = (n * AN) // 128 + jj
                        nc.vector.tensor_copy(vt_all[:, jc_g, :], pv[:])

            # ---------- Phase B: causal attention per head ----------
            bc_ctx = tc.tile_pool(name="bconst", bufs=1)
            bcpool = bc_ctx.__enter__()
            ao_ctx = tc.tile_pool(name="ao", bufs=1)
            aopool = ao_ctx.__enter__()
            cw_ctx = tc.tile_pool(name="cw", bufs=1)
            cwpool = cw_ctx.__enter__()
            mask_sb = bcpool.tile([128, HPC, SC], F32R, name="mask_sb")
            nc.sync.dma_start(mask_sb[:], masks[:].rearrange("t p c -> p t c"))
            ones_sb = bcpool.tile([128, 128], F32R, name="ones_sb")
            nc.sync.dma_start(ones_sb[:], ones[:])

            ao_tiles = []
            wo_sb = cwpool.tile([128, HPC, D], F32R, name="wo_sb")
            with (
                tc.tile_pool(name="bkv", bufs=2) as bkv,
                tc.tile_pool(name="bp", bufs=6) as bp,
                tc.tile_pool(name="bli", bufs=2) as bli,
                tc.tile_pool(name="bps_s", bufs=4, space="PSUM") as bps_s,
                tc.tile_pool(name="bps_o", bufs=2, space="PSUM") as bps_o,
                tc.tile_pool(name="bps_l", bufs=2, space="PSUM") as bps_l,
            ):
                for h0 in range(HPC):
                    ktr = bkv.tile([DK, S], F32R, tag="ktr")
                    for si in range(NI):
                        sl = slice(si * SC, (si + 1) * SC)
                        nc.sync.dma_start(ktr[:, sl], k_d[h0][:, sl])
                    qtr = bkv.tile([DK, S], F32R, tag="qtr")
                    for si in range(NI):
                        sl = slice(si * SC, (si + 1) * SC)
                        nc.sync.dma_start(qtr[:, sl], q_d[h0][:, sl])
                    if h0 == 0:
                        # prefetch the output-projection weights during B
                        nc.sync.dma_start(
                            wo_sb[:],
                            woT[:].rearrange("(ec p) f -> p ec f", p=128),
                        )

                    ao_t = aopool.tile([DK, S], F32R, name=f"ao_{h0}")
                    ao_tiles.append(ao_t)

                    for ic in range(NI):
                        po = bps_o.tile([128, SC], F32, tag="po")
                        pl = bps_l.tile([128, SC], F32, tag="pl")
                        njc = 4 * ic + 4
                        i0 = ic * SC

                        def emit_pv(p, jc, cs):
                            nc.tensor.matmul(
                                po[:, cs:],
                                vt_all[:, jc, h0 * DK : (h0 + 1) * DK],
                                p[:, cs:],
                                start=(jc == 0),
                                stop=(jc == njc - 1),
                            )
                            nc.tensor.matmul(
                                pl[:, cs:],
                                ones_sb[:],
                                p[:, cs:],
                                start=(jc == 0),
                                stop=(jc == njc - 1),
                            )

                        pending = []
                        for jc in range(njc):
                            t = jc - 4 * ic  # >=0 on the causal diagonal band
                            cs = 128 * t if t >= 0 else 0
                            ps = bps_s.tile([128, SC], F32, tag="ps")
                            nc.tensor.matmul(
                                ps[:, cs:],
                                ktr[:, jc * 128 : (jc + 1) * 128],
                                qtr[:, i0 + cs : i0 + SC],
                                start=True,
                                stop=True,
                            )
                            p = bp.tile([128, SC], F32R, tag="p")
                            nc.scalar.activation(
                                p[:, cs:], ps[:, cs:], AF.Exp, scale=float(ISQRT_DK)
                            )
                            if t >= 0:
                                nc.vector.tensor_mul(
                                    p[:, cs : cs + 128],
                                    p[:, cs : cs + 128],
                                    mask_sb[:, t, cs : cs + 128],
                                )
                            # software pipeline: scores run up to two tiles
                            # ahead of the P@V / row-sum matmuls so the ACT
                            # exp latency stays off the tensor-engine path
                            pending.append((p, jc, cs))
                            if len(pending) > 2:
                                emit_pv(*pending.pop(0))
                        for it in pending:
                            emit_pv(*it)

                        li = bli.tile([128, SC], F32, tag="li")
                        nc.vector.reciprocal_approx_fast(li[:], pl[:])
                        nc.vector.tensor_mul(
                            ao_t[:, i0 : i0 + SC], po[:], li[:]
                        )

            # ---------- Phase C: output projection (partial sum) ----------
            with (
                tc.tile_pool(name="cst", bufs=6) as cst,
                tc.tile_pool(name="cps", bufs=8, space="PSUM") as cps,
            ):
                for ii in range(S // 128):
                    pcs = [
                        cps.tile([128, 512], F32, tag="pc", name=f"pc_{ii}_{fc}")
                        for fc in range(4)
                    ]
                    for ec in range(HPC):
                        for fc in range(4):
                            nc.tensor.matmul(
                                pcs[fc][:],
                                ao_tiles[ec][:, ii * 128 : (ii + 1) * 128],
                                wo_sb[:, ec, fc * 512 : (fc + 1) * 512],
                                start=(ec == 0),
                                stop=(ec == HPC - 1),
                            )
                    for fc in range(4):
                        ob = cst.tile([128, 512], F32, tag="ob")
                        if fc % 2 == 0:
                            nc.vector.tensor_copy(ob[:], pcs[fc][:])
                        else:
                            nc.scalar.activation(ob[:], pcs[fc][:], AF.Copy)
                        nc.sync.dma_start(
                            out[ii * 128 : (ii + 1) * 128, fc * 512 : (fc + 1) * 512],
                            ob[:],
                        )

            cw_ctx.__exit__(None, None, None)
            ao_ctx.__exit__(None, None, None)
            bc_ctx.__exit__(None, None, None)
            vres_ctx.__exit__(None, None, None)

    nc.compile()
    return nc


def _rope_tables():
    inv_freq = 1.0 / (10000.0 ** (np.arange(0, DK, 2, dtype=np.float64) / DK))
    pos = np.arange(S, dtype=np.float64)
    freqs = pos[:, None] * inv_freq[None, :]  # [S, DK/2]
    cos_t = np.cos(freqs).T.astype(np.float32)  # [64, S]
    sin_t = np.sin(freqs).T.astype(np.float32)
    cc2 = np.ascontiguousarray(np.concatenate([cos_t, cos_t], axis=0))
    sss = np.ascontiguousarray(np.concatenate([-sin_t, sin_t], axis=0))
    return cc2, sss


def kernel(
    x, wq_w, wq_b, wk_w, wk_b, wv_w, wv_b, wo_w, wo_b
) -> np.ndarray:
    global last_exec_time_ns, last_results
    from concourse.bass_utils import run_bass_kernel_spmd

    if "nc" not in _CACHE:
        _CACHE["nc"] = _build_program()
    nc = _CACHE["nc"]

    x = np.asarray(x, dtype=np.float32)
    wq_w = np.asarray(wq_w, dtype=np.float32)
    wk_w = np.asarray(wk_w, dtype=np.float32)
    wv_w = np.asarray(wv_w, dtype=np.float32)
    wo_w = np.asarray(wo_w, dtype=np.float32)
    wq_b = np.asarray(wq_b, dtype=np.float32)
    wk_b = np.asarray(wk_b, dtype=np.float32)
    wv_b = np.asarray(wv_b, dtype=np.float32)
    wo_b = np.asarray(wo_b, dtype=np.float32)

    cc2, sss = _rope_tables()
    r_idx = np.arange(128)[:, None]
    c_idx = np.arange(SC)[None, :]
    masks = np.ascontiguousarray(
        np.stack(
            [(r_idx <= c_idx - t * 128).astype(np.float32) for t in range(HPC)]
        )
    )
    ones = np.ones((128, 128), dtype=np.float32)
    # within each head, pack d-rows as [even dims; odd dims]
    perm = np.concatenate([np.arange(0, DK, 2), np.arange(1, DK, 2)])

    xT_b = [np.ascontiguousarray(x[b].T) for b in range(B)]

    in_maps = []
    for c in range(N_CORES):
        b = c // (N_CORES // B)
        g = c % (N_CORES // B)
        es = g * E

        def pack_qk(w):
            rows = w[es : es + E]  # [E, D]
            blocks = [
                rows[h0 * DK : (h0 + 1) * DK][perm] for h0 in range(HPC)
            ]
            return np.ascontiguousarray(np.concatenate(blocks, axis=0).T)

        def pack_bias(bvec):
            sl = bvec[es : es + E].reshape(HPC, DK)
            return np.ascontiguousarray(sl[:, perm])

        in_maps.append(
            {
                "xT": xT_b[b],
                "wqT": pack_qk(wq_w),
                "wkT": pack_qk(wk_w),
                "wvT": np.ascontiguousarray(wv_w[es : es + E].T),
                "woT": np.ascontiguousarray(wo_w[:, es : es + E].T),
                "bq": pack_bias(wq_b),
                "bk": pack_bias(wk_b),
                "cc2": cc2,
                "sss": sss,
                "masks": masks,
                "ones": ones,
            }
        )

    trace = bool(os.environ.get("MHA_TRACE"))
    res = run_bass_kernel_spmd(
        nc, in_maps, list(range(N_CORES)), trace=trace
    )
    last_exec_time_ns = res.exec_time_ns
    last_results = res

    # host-side gather: sum partials per batch, add biases that commute
    # with attention (softmax rows sum to 1, so wv_b passes straight
    # through to the output projection)
    const_bias = wo_b + wo_w @ wv_b  # [D]
    out = np.empty((B, S, D), dtype=np.float32)
    gpb = N_CORES // B
    for b in range(B):
        acc = res.results[b * gpb]["out"].copy()
        for c in range(b * gpb + 1, (b + 1) * gpb):
            acc += res.results[c]["out"]
        out[b] = acc + const_bias[None, :]
    return out



# revision 3
# speedup vs baseline: 1.1763x; 1.1763x over previous
"""Trainium2 Bass kernel for causal multi-head attention with RoPE.

Full-input contract: kernel(**inputs) takes the unsharded tensors and
returns the full [B, S, D] output. Internally the work is sharded over
8 NeuronCores: cores 0-3 compute batch 0, cores 4-7 batch 1; within a
batch group each core owns 4 of the 16 heads (tensor-parallel over
heads). Each core computes its partial output-projection contribution
[S, D]; the host sums the 4 partials per batch and adds the biases
that commute with attention (wo_b, and wv_b which passes through the
softmax untouched because attention weights sum to 1).

The on-device data path is bf16 (fp32 PSUM accumulation): same PE
rate as fp32r (1 row/cycle) but half the DMA/SBUF footprint, which
lets Q/K/V stay SBUF-resident between the projection and attention
phases (no DRAM round-trip) and doubles vector-engine throughput.
"""

import os
import sys

sys.path.insert(0, "/opt/trn_rl_repo")

import numpy as np
import ml_dtypes

BF16 = ml_dtypes.bfloat16

B = 2
S = 2048
D = 2048
H = 16
DK = 128
N_CORES = 8
HPC = 4          # heads per core
E = HPC * DK     # 512: per-core slice of the model dim
AN = 512         # phase-A sequence chunk (moving free dim for Q/K)
SC = 512         # attention query chunk (moving free dim)
KO = D // 128    # contraction chunks for the projections
NJ = S // 128    # key chunks
NI = S // SC     # query chunks
NN = S // AN     # phase-A chunks
ISQRT_DK = 1.0 / np.sqrt(DK)

_CACHE = {}

last_exec_time_ns = None
last_results = None


def _build_program():
    import concourse.mybir as mybir
    import concourse.tile as tile
    from concourse import bacc

    dt = mybir.dt
    F32 = dt.float32
    BF = dt.bfloat16
    AF = mybir.ActivationFunctionType

    nc = bacc.Bacc(None, target_bir_lowering=False, debug=True)

    xT = nc.dram_tensor("xT", [D, S], BF, kind="ExternalInput")
    wqT = nc.dram_tensor("wqT", [D, E], BF, kind="ExternalInput")
    wkT = nc.dram_tensor("wkT", [D, E], BF, kind="ExternalInput")
    wvT = nc.dram_tensor("wvT", [D, E], BF, kind="ExternalInput")
    woT = nc.dram_tensor("woT", [E, D], BF, kind="ExternalInput")
    bq = nc.dram_tensor("bq", [HPC, DK], F32, kind="ExternalInput")
    bk = nc.dram_tensor("bk", [HPC, DK], F32, kind="ExternalInput")
    cc2 = nc.dram_tensor("cc2", [DK, S], BF, kind="ExternalInput")
    sss = nc.dram_tensor("sss", [DK, S], BF, kind="ExternalInput")
    masks = nc.dram_tensor("masks", [HPC, 128, SC], BF, kind="ExternalInput")
    ones = nc.dram_tensor("ones", [128, 128], BF, kind="ExternalInput")
    out = nc.dram_tensor("out", [S, D], F32, kind="ExternalOutput")

    with tile.TileContext(nc) as tc:
        with tc.tile_pool(name="const", bufs=1) as cpool:
            bq_sb = cpool.tile([DK, HPC], F32, name="bq_sb")
            nc.sync.dma_start(bq_sb[:], bq[:].rearrange("h d -> d h"))
            bk_sb = cpool.tile([DK, HPC], F32, name="bk_sb")
            nc.sync.dma_start(bk_sb[:], bk[:].rearrange("h d -> d h"))
            cc2_sb = cpool.tile([DK, S], BF, name="cc2_sb")
            nc.gpsimd.dma_start(cc2_sb[:], cc2[:])
            sss_sb = cpool.tile([DK, S], BF, name="sss_sb")
            nc.gpsimd.dma_start(sss_sb[:], sss[:])
            mask_sb = cpool.tile([128, HPC, SC], BF, name="mask_sb")
            nc.gpsimd.dma_start(mask_sb[:], masks[:].rearrange("t p c -> p t c"))
            ones_sb = cpool.tile([128, 128], BF, name="ones_sb")
            nc.gpsimd.dma_start(ones_sb[:], ones[:])

            # persistent activations: V, Q, K, attention output (all bf16)
            res_ctx = tc.tile_pool(name="resident", bufs=1)
            rpool = res_ctx.__enter__()
            vt_all = rpool.tile([128, NJ, E], BF, name="vt_all")
            q_all = rpool.tile([DK, HPC, S], BF, name="q_all")
            k_all = rpool.tile([DK, HPC, S], BF, name="k_all")
            ao_all = rpool.tile([DK, HPC, S], BF, name="ao_all")
            wo_sb = rpool.tile([128, HPC, D], BF, name="wo_sb")
            # output-projection weights stream on the gpsimd queue; they are
            # only needed in phase C
            nc.gpsimd.dma_start(
                wo_sb[:], woT[:].rearrange("(ec p) f -> p ec f", p=128)
            )

            # ---------- Phase A: Q/K/V projections (+ RoPE on Q/K) ----------
            with (
                tc.tile_pool(name="aw", bufs=1) as awpool,
                tc.tile_pool(name="ax", bufs=2) as axpool,
                tc.tile_pool(name="ast", bufs=3) as astpool,
                tc.tile_pool(name="aps0", bufs=4, space="PSUM") as aps0,
                tc.tile_pool(name="aps", bufs=2, space="PSUM") as apspool,
            ):
                # weights and x split into per-g tiles so the first matmuls
                # wait only on the first 512-row piece, not the whole tensor
                def load_w(wdram, nm):
                    tiles = []
                    for g in range(4):
                        t = awpool.tile([128, 4, E], BF, name=f"{nm}{g}")
                        nc.sync.dma_start(
                            t[:],
                            wdram[g * 512 : (g + 1) * 512, :].rearrange(
                                "(ko p) m -> p ko m", p=128
                            ),
                        )
                        tiles.append(t)
                    return tiles

                def load_xn(n):
                    tiles = []
                    for g in range(4):
                        t = axpool.tile(
                            [128, 4, AN], BF, tag=f"xn{g}", name=f"xn{n}_{g}"
                        )
                        nc.sync.dma_start(
                            t[:],
                            xT[
                                g * 512 : (g + 1) * 512,
                                n * AN : (n + 1) * AN,
                            ].rearrange("(ko p) s -> p ko s", p=128),
                        )
                        tiles.append(t)
                    return tiles

                # need-ordered input stream on the sync queue: interleave
                # (wq, x0) pieces, then wk, then wv; later x chunks follow
                wq_t = []
                x_next = []
                for g in range(4):
                    t = awpool.tile([128, 4, E], BF, name=f"wq{g}")
                    nc.sync.dma_start(
                        t[:],
                        wqT[g * 512 : (g + 1) * 512, :].rearrange(
                            "(ko p) m -> p ko m", p=128
                        ),
                    )
                    wq_t.append(t)
                    xt = axpool.tile([128, 4, AN], BF, tag=f"xn{g}", name=f"xn0_{g}")
                    nc.sync.dma_start(
                        xt[:],
                        xT[g * 512 : (g + 1) * 512, 0:AN].rearrange(
                            "(ko p) s -> p ko s", p=128
                        ),
                    )
                    x_next.append(xt)
                wk_t = load_w(wkT, "wk")
                wv_t = load_w(wvT, "wv")

                def rope_store(pq, bsb, m, dst, nsl):
                    st0 = astpool.tile([128, AN], BF, tag="qkst0")
                    nc.scalar.activation(
                        st0[:], pq[:], AF.Identity, bias=bsb[:, m : m + 1]
                    )
                    # RoPE: d-rows are packed [even; odd] per head, so
                    # rotate pairs are partition r <-> r+64
                    sw = astpool.tile([128, AN], BF, tag="qksw")
                    nc.vector.tensor_copy(sw[0:64, :], st0[64:128, :])
                    nc.vector.tensor_copy(sw[64:128, :], st0[0:64, :])
                    rot = astpool.tile([128, AN], BF, tag="qkrot")
                    nc.vector.tensor_mul(rot[:], st0[:], cc2_sb[:, nsl])
                    nc.vector.tensor_mul(sw[:], sw[:], sss_sb[:, nsl])
                    nc.vector.tensor_add(dst[:, m, nsl], rot[:], sw[:])

                for n in range(NN):
                    xn = x_next
                    if n + 1 < NN:
                        x_next = load_xn(n + 1)
                    nsl = slice(n * AN, (n + 1) * AN)
                    # Q and K: out[d, s], then bias + RoPE (result written
                    # straight into the resident SBUF q/k tiles).
                    # First chunk runs k-outer so the PE consumes each
                    # weight/x piece as it arrives instead of stalling.
                    for wt, bsb, dst in ((wq_t, bq_sb, q_all), (wk_t, bk_sb, k_all)):
                        if n == 0:
                            pqs = [
                                aps0.tile(
                                    [128, AN], F32, tag="pqk0", name=f"pq0_{m}"
                                )
                                for m in range(HPC)
                            ]
                            for k in range(KO):
                                for m in range(HPC):
                                    nc.tensor.matmul(
                                        pqs[m][:],
                                        wt[k // 4][:, k % 4, m * DK : (m + 1) * DK],
                                        xn[k // 4][:, k % 4, :],
                                        start=(k == 0),
                                        stop=(k == KO - 1),
                                    )
                            for m in range(HPC):
                                rope_store(pqs[m], bsb, m, dst, nsl)
                        else:
                            for m in range(HPC):
                                pq = apspool.tile([128, AN], F32, tag="pqk")
                                for k in range(KO):
                                    nc.tensor.matmul(
                                        pq[:],
                                        wt[k // 4][:, k % 4, m * DK : (m + 1) * DK],
                                        xn[k // 4][:, k % 4, :],
                                        start=(k == 0),
                                        stop=(k == KO - 1),
                                    )
                                rope_store(pq, bsb, m, dst, nsl)
                    # V: out[s, d] with s on partitions (natural for P@V)
                    for jj in range(AN // 128):
                        pv = apspool.tile([128, E], F32, tag="pv")
                        for k in range(KO):
                            nc.tensor.matmul(
                                pv[:],
                                xn[k // 4][:, k % 4, jj * 128 : (jj + 1) * 128],
                                wv_t[k // 4][:, k % 4, :],
                                start=(k == 0),
                                stop=(k == KO - 1),
                            )
                        nc.vector.tensor_copy(vt_all[:, n * 4 + jj, :], pv[:])

            # ---------- Phase B: causal attention per head ----------
            # scores land in paired PSUM tiles [128, 2, SC] so one exp
            # instruction covers two key-chunks (amortizes ACT overhead);
            # the softmax row-sum rides the tensor engine (ones matmul).
            # A software pipeline carried across (head, ic) iterations keeps
            # the PE from draining at chunk boundaries.
            with (
                tc.tile_pool(name="bp", bufs=6) as bp,
                tc.tile_pool(name="bli", bufs=2) as bli,
                tc.tile_pool(name="bps_s", bufs=2, space="PSUM") as bps_s,
                tc.tile_pool(name="bps_o", bufs=2, space="PSUM") as bps_o,
                tc.tile_pool(name="bps_l", bufs=2, space="PSUM") as bps_l,
            ):
                pending = []  # (p2, half, jc, cs, po, pl, njc, fin)
                DEPTH = 3

                def emit_pv(p2, half, jc, cs, po, pl, njc, fin):
                    h0 = fin[0]
                    nc.tensor.matmul(
                        po[:, cs:],
                        vt_all[:, jc, h0 * DK : (h0 + 1) * DK],
                        p2[:, half, cs:],
                        start=(jc == 0),
                        stop=(jc == njc - 1),
                    )
                    nc.tensor.matmul(
                        pl[:, cs:],
                        ones_sb[:],
                        p2[:, half, cs:],
                        start=(jc == 0),
                        stop=(jc == njc - 1),
                    )
                    if jc == njc - 1:
                        # normalization for this (head, ic) now that the
                        # last accumulating matmul is emitted
                        _, i0 = fin
                        li = bli.tile([128, SC], F32, tag="li")
                        nc.vector.reciprocal_approx_fast(li[:], pl[:])
                        nc.vector.tensor_mul(
                            ao_all[:, h0, i0 : i0 + SC], po[:], li[:]
                        )

                for h0 in range(HPC):
                    for ic in range(NI):
                        po = bps_o.tile([128, SC], F32, tag="po")
                        pl = bps_l.tile([128, SC], F32, tag="pl")
                        njc = 4 * ic + 4
                        i0 = ic * SC
                        fin = (h0, i0)

                        for jp in range(njc // 2):
                            ps = bps_s.tile([128, 2, SC], F32, tag="ps")
                            p2 = bp.tile([128, 2, SC], BF, tag="p")
                            css = []
                            for half in range(2):
                                jc = 2 * jp + half
                                t = jc - 4 * ic  # >=0 on the diagonal band
                                cs = 128 * t if t >= 0 else 0
                                css.append((jc, t, cs))
                                nc.tensor.matmul(
                                    ps[:, half, cs:],
                                    k_all[:, h0, jc * 128 : (jc + 1) * 128],
                                    q_all[:, h0, i0 + cs : i0 + SC],
                                    start=True,
                                    stop=True,
                                )
                            t0_, t1_ = css[0][1], css[1][1]
                            if t1_ <= 1:
                                # both halves (nearly) full: one wide exp
                                nc.scalar.activation(
                                    p2[:], ps[:], AF.Exp, scale=float(ISQRT_DK)
                                )
                            else:
                                for half, (jc, t, cs) in enumerate(css):
                                    nc.scalar.activation(
                                        p2[:, half, cs:],
                                        ps[:, half, cs:],
                                        AF.Exp,
                                        scale=float(ISQRT_DK),
                                    )
                            for half, (jc, t, cs) in enumerate(css):
                                if t >= 0:
                                    nc.vector.tensor_mul(
                                        p2[:, half, cs : cs + 128],
                                        p2[:, half, cs : cs + 128],
                                        mask_sb[:, t, cs : cs + 128],
                                    )
                                pending.append((p2, half, jc, cs, po, pl, njc, fin))
                                if len(pending) > DEPTH:
                                    emit_pv(*pending.pop(0))
                for it in pending:
                    emit_pv(*it)
                pending.clear()

            # ---------- Phase C: output projection (partial sum) ----------
            with (
                tc.tile_pool(name="cst", bufs=6) as cst,
                tc.tile_pool(name="cps", bufs=8, space="PSUM") as cps,
            ):
                for ii in range(S // 128):
                    pcs = [
                        cps.tile([128, 512], F32, tag="pc", name=f"pc_{ii}_{fc}")
                        for fc in range(4)
                    ]
                    for ec in range(HPC):
                        for fc in range(4):
                            nc.tensor.matmul(
                                pcs[fc][:],
                                ao_all[:, ec, ii * 128 : (ii + 1) * 128],
                                wo_sb[:, ec, fc * 512 : (fc + 1) * 512],
                                start=(ec == 0),
                                stop=(ec == HPC - 1),
                            )
                    for fc in range(4):
                        ob = cst.tile([128, 512], F32, tag="ob")
                        if fc % 2 == 0:
                            nc.vector.tensor_copy(ob[:], pcs[fc][:])
                        else:
                            nc.scalar.activation(ob[:], pcs[fc][:], AF.Copy)
                        q = nc.sync if fc % 2 == 0 else nc.scalar
                        q.dma_start(
                            out[ii * 128 : (ii + 1) * 128, fc * 512 : (fc + 1) * 512],
                            ob[:],
                        )

            res_ctx.__exit__(None, None, None)

    nc.compile()
    return nc


def _rope_tables():
    inv_freq = 1.0 / (10000.0 ** (np.arange(0, DK, 2, dtype=np.float64) / DK))
    pos = np.arange(S, dtype=np.float64)
    freqs = pos[:, None] * inv_freq[None, :]  # [S, DK/2]
    cos_t = np.cos(freqs).T.astype(np.float32)  # [64, S]
    sin_t = np.sin(freqs).T.astype(np.float32)
    cc2 = np.ascontiguousarray(np.concatenate([cos_t, cos_t], axis=0))
    sss = np.ascontiguousarray(np.concatenate([-sin_t, sin_t], axis=0))
    return cc2, sss


def kernel(
    x, wq_w, wq_b, wk_w, wk_b, wv_w, wv_b, wo_w, wo_b
) -> np.ndarray:
    global last_exec_time_ns, last_results
    from concourse.bass_utils import run_bass_kernel_spmd

    if "nc" not in _CACHE:
        _CACHE["nc"] = _build_program()
    nc = _CACHE["nc"]

    x = np.asarray(x, dtype=np.float32)
    wq_w = np.asarray(wq_w, dtype=np.float32)
    wk_w = np.asarray(wk_w, dtype=np.float32)
    wv_w = np.asarray(wv_w, dtype=np.float32)
    wo_w = np.asarray(wo_w, dtype=np.float32)
    wq_b = np.asarray(wq_b, dtype=np.float32)
    wk_b = np.asarray(wk_b, dtype=np.float32)
    wv_b = np.asarray(wv_b, dtype=np.float32)
    wo_b = np.asarray(wo_b, dtype=np.float32)

    cc2, sss = _rope_tables()
    r_idx = np.arange(128)[:, None]
    c_idx = np.arange(SC)[None, :]
    masks = np.ascontiguousarray(
        np.stack(
            [(r_idx <= c_idx - t * 128).astype(np.float32) for t in range(HPC)]
        )
    ).astype(BF16)
    ones = np.ones((128, 128), dtype=BF16)
    # within each head, pack d-rows as [even dims; odd dims]
    perm = np.concatenate([np.arange(0, DK, 2), np.arange(1, DK, 2)])

    xT_b = [np.ascontiguousarray(x[b].T).astype(BF16) for b in range(B)]
    cc2 = cc2.astype(BF16)
    sss = sss.astype(BF16)

    in_maps = []
    for c in range(N_CORES):
        b = c // (N_CORES // B)
        g = c % (N_CORES // B)
        es = g * E

        def pack_qk(w):
            rows = w[es : es + E]  # [E, D]
            blocks = [
                rows[h0 * DK : (h0 + 1) * DK][perm] for h0 in range(HPC)
            ]
            return np.ascontiguousarray(np.concatenate(blocks, axis=0).T).astype(
                BF16
            )

        def pack_bias(bvec):
            sl = bvec[es : es + E].reshape(HPC, DK)
            return np.ascontiguousarray(sl[:, perm])

        in_maps.append(
            {
                "xT": xT_b[b],
                "wqT": pack_qk(wq_w),
                "wkT": pack_qk(wk_w),
                "wvT": np.ascontiguousarray(wv_w[es : es + E].T).astype(BF16),
                "woT": np.ascontiguousarray(wo_w[:, es : es + E].T).astype(BF16),
                "bq": pack_bias(wq_b),
                "bk": pack_bias(wk_b),
                "cc2": cc2,
                "sss": sss,
                "masks": masks,
                "ones": ones,
            }
        )

    trace = bool(os.environ.get("MHA_TRACE"))
    res = run_bass_kernel_spmd(
        nc, in_maps, list(range(N_CORES)), trace=trace
    )
    last_exec_time_ns = res.exec_time_ns
    last_results = res

    # host-side gather: sum partials per batch, add biases that commute
    # with attention (softmax rows sum to 1, so wv_b passes straight
    # through to the output projection)
    const_bias = wo_b + wo_w @ wv_b  # [D]
    out = np.empty((B, S, D), dtype=np.float32)
    gpb = N_CORES // B
    for b in range(B):
        acc = res.results[b * gpb]["out"].copy()
        for c in range(b * gpb + 1, (b + 1) * gpb):
            acc += res.results[c]["out"]
        out[b] = acc + const_bias[None, :]
    return out
